# revision 10
# baseline (speedup 1.0000x reference)
"""Trainium2 Bass kernel for DilatedReparamConv (5-branch depthwise conv + BN + SiLU + identity BN).

out = BN_id(x) + sum_i silu(BN_i(dwconv_i(x)))   for branches
      (5,d1), (7,d2), (3,d3), (3,d4), (3,d5), all SAME padding.

Strategy (8 NeuronCores, SPMD):
  - Shard the 256 channels across 8 cores (32 ch/core, all 32 images).
  - Depthwise conv on TensorE: for each channel-pair, a block-diagonal banded
    Toeplitz matrix T[(ci,h_in),(ci,h_out)] contracts the whole kh tap-stack in
    one matmul; kw taps become free-dim shifts into a W-padded x tile and
    accumulate in PSUM (one matmul per kw tap, start/stop flags).
  - BN+SiLU fused into the ScalarE PSUM->SBUF eviction (per-partition
    scale/bias APs). Branch accumulation on VectorE. Identity branch on
    ScalarE (Identity activation with scale/bias).
  - Host precomputes the banded matrices and a [C, H, B, Wpad] padded layout
    so every DMA is a contiguous block.
"""

import sys

sys.path.insert(0, "/opt/trn_rl_repo")

import numpy as np
import ml_dtypes

import concourse.bass as bass
import concourse.mybir as mybir
from concourse import bacc, tile
from concourse.bass_utils import run_bass_kernel_spmd

# ---------------------------------------------------------------- problem dims
B, C, H, W = 32, 256, 64, 64
EPS = 1e-5
BRANCH_CFG = [(5, 1), (7, 2), (3, 3), (3, 4), (3, 5)]  # (kernel, dilation)
N_CORES = 8
C_CORE = C // N_CORES          # 32 channels per core
PAIRS = C_CORE // 2            # 16 channel-pairs per core
PAD = 6                        # max dilation*(ks-1)//2 across branches
WP = W + 2 * PAD               # padded width = 76
NTAPS = sum(ks for ks, _ in BRANCH_CFG)   # 21 kw taps total
IMG_CHUNK = 8                  # images per matmul chunk (8*64 = 512 free)
N_CHUNKS = B // IMG_CHUNK      # 4

# matmul input dtype: "bf16" | "f32r" | "f32"
MM_DTYPE = "bf16"

_CACHE: dict = {}


def _mm_np_dtype():
    return ml_dtypes.bfloat16 if MM_DTYPE == "bf16" else np.float32


def _mm_bir_dtype():
    return mybir.dt.bfloat16 if MM_DTYPE == "bf16" else mybir.dt.float32


def _taps():
    """Yield (j, branch_idx, dx) for the 21 kw taps, branch-major."""
    out = []
    j = 0
    for br, (ks, dil) in enumerate(BRANCH_CFG):
        pad = dil * (ks - 1) // 2
        for kw in range(ks):
            out.append((j, br, dil * kw - pad))
            j += 1
    return out


def build_nc():
    """Build the single-core Bass program (same for all 8 cores)."""
    nc = bacc.Bacc(
        "TRN2", target_bir_lowering=False, debug=False, num_devices=N_CORES
    )

    dt_in = _mm_bir_dtype()
    f32 = mybir.dt.float32

    # x (padded) and the 21 Toeplitz matrices concatenated per pair: one DMA,
    # one semaphore per consumer. scale|bias merged likewise.
    XCOLS = B * WP
    XWCOLS = XCOLS + NTAPS * 128
    xw = nc.dram_tensor("xw", [PAIRS, 128, XWCOLS], dt_in, kind="ExternalInput").ap()
    scbi = nc.dram_tensor("scbi", [128, 2 * PAIRS * 6], f32, kind="ExternalInput").ap()
    yt = nc.dram_tensor("yt", [PAIRS, 128, B * W], f32, kind="ExternalOutput").ap()

    taps = _taps()
    br_tap_ranges = []
    j0 = 0
    for ks, _ in BRANCH_CFG:
        br_tap_ranges.append((j0, j0 + ks))
        j0 += ks

    with tile.TileContext(nc) as tc:
        with (
            tc.tile_pool(name="consts", bufs=1) as consts,
            tc.tile_pool(name="xwp", bufs=3) as xwp,
            tc.tile_pool(name="accp", bufs=2) as accp,
            tc.tile_pool(name="tp", bufs=3) as tp,
            tc.tile_pool(name="psum", bufs=6, space="PSUM") as psum,
        ):
            scbi_t = consts.tile([128, 2 * PAIRS * 6], f32)
            nc.sync.dma_start(out=scbi_t[:], in_=scbi)
            sc_t = scbi_t[:, : PAIRS * 6]
            bi_t = scbi_t[:, PAIRS * 6 :]

            for p in range(PAIRS):
                xw_t = xwp.tile([128, XWCOLS], dt_in)
                nc.sync.dma_start(out=xw_t[:], in_=xw[p])
                xt_t = xw_t[:, :XCOLS]
                wt_t = xw_t[:, XCOLS:]

                acc = accp.tile([128, B * W], f32)
                xr = xt_t.rearrange("p (b w) -> p b w", w=WP)

                # identity branch: acc = x * s_id + b_id  (ScalarE)
                acc3 = acc.rearrange("p (b w) -> p b w", w=W)
                nc.scalar.activation(
                    acc3[:, :, :],
                    xr[:, :, PAD : PAD + W],
                    mybir.ActivationFunctionType.Identity,
                    bias=bi_t[:, p * 6 + 5 : p * 6 + 6],
                    scale=sc_t[:, p * 6 + 5 : p * 6 + 6],
                )

                for br in range(5):
                    jlo, jhi = br_tap_ranges[br]
                    t_full = tp.tile([128, B * W], f32)
                    for cch in range(N_CHUNKS):
                        ps = psum.tile([128, IMG_CHUNK * W], f32)
                        b0 = cch * IMG_CHUNK
                        for j, _br, dx in taps[jlo:jhi]:
                            rhs = xr[:, b0 : b0 + IMG_CHUNK, PAD + dx : PAD + dx + W]
                            lhsT = wt_t[:, j * 128 : (j + 1) * 128]
                            nc.tensor.matmul(
                                ps[:],
                                lhsT,
                                rhs,
                                start=(j == jlo),
                                stop=(j == jhi - 1),
                            )
                        # silu(BN(psum)) -> t chunk  (ScalarE)
                        nc.scalar.activation(
                            t_full[:, cch * IMG_CHUNK * W : (cch + 1) * IMG_CHUNK * W],
                            ps[:],
                            mybir.ActivationFunctionType.Silu,
                            bias=bi_t[:, p * 6 + br : p * 6 + br + 1],
                            scale=sc_t[:, p * 6 + br : p * 6 + br + 1],
                        )
                    # acc += t  (VectorE, one big add per branch)
                    nc.vector.tensor_tensor(
                        acc[:], acc[:], t_full[:], op=mybir.AluOpType.add
                    )

                nc.sync.dma_start(out=yt[p], in_=acc[:])

    nc.compile()
    return nc


# ------------------------------------------------------------------ host prep
def _bn_scale_bias(gamma, beta, mean, var):
    s = gamma / np.sqrt(var + EPS)
    return s, beta - mean * s


def _host_prep(x, id_bn, w5, w7, w3a, w3b, w3c, bn_gamma, bn_beta, bn_mean, bn_var):
    x = np.asarray(x, np.float32)
    weights = [np.asarray(w, np.float32) for w in (w5, w7, w3a, w3b, w3c)]
    mmdt = _mm_np_dtype()

    # x -> [C, H, B, Wpad], pad W by 6 zeros each side
    xt_full = np.zeros((C, H, B, WP), np.float32)
    xt_full[:, :, :, PAD : PAD + W] = np.transpose(x, (1, 2, 0, 3))
    # per-core: [PAIRS, 2, 64, B, WP] -> [PAIRS, 128, B*WP]
    xt_cores = []
    for k in range(N_CORES):
        xs = xt_full[k * C_CORE : (k + 1) * C_CORE]
        xs = xs.reshape(PAIRS, 2 * H, B * WP).astype(mmdt)
        xt_cores.append(np.ascontiguousarray(xs))

    # Toeplitz bands: T[c, j, hi, ho]
    T = np.zeros((C, NTAPS, H, H), np.float32)
    j = 0
    for br, (ks, dil) in enumerate(BRANCH_CFG):
        pad = dil * (ks - 1) // 2
        wbr = weights[br][:, 0]  # [C, ks, ks]
        for kw in range(ks):
            for kh in range(ks):
                off = dil * kh - pad
                ho = np.arange(max(0, -off), min(H, H - off))
                T[:, j, ho + off, ho] = wbr[:, kh, kw][:, None]
            j += 1

    # per-core wm: [PAIRS, 128(K), NTAPS, 128(M)] block-diag over ci
    Tr = T.reshape(N_CORES, PAIRS, 2, NTAPS, H, H)
    wm_cores = []
    for k in range(N_CORES):
        wmk = np.zeros((PAIRS, 128, NTAPS, 128), np.float32)
        for ci in range(2):
            # [pair, j, hi, ho] -> [pair, hi(K), j, ho(M)]
            wmk[:, ci * H : (ci + 1) * H, :, ci * H : (ci + 1) * H] = np.transpose(
                Tr[k, :, ci], (0, 2, 1, 3)
            )
        wm_cores.append(
            np.ascontiguousarray(wmk.reshape(PAIRS, 128, NTAPS * 128).astype(mmdt))
        )

    # BN scale/bias rows: branches 0-4, identity = 5
    S = np.zeros((6, C), np.float32)
    Bv = np.zeros((6, C), np.float32)
    bn_gamma = np.asarray(bn_gamma, np.float32)
    bn_beta = np.asarray(bn_beta, np.float32)
    bn_mean = np.asarray(bn_mean, np.float32)
    bn_var = np.asarray(bn_var, np.float32)
    id_bn = np.asarray(id_bn, np.float32)
    for i in range(5):
        S[i], Bv[i] = _bn_scale_bias(bn_gamma[i], bn_beta[i], bn_mean[i], bn_var[i])
    S[5], Bv[5] = _bn_scale_bias(id_bn[0], id_bn[1], id_bn[2], id_bn[3])

    scbi_cores = []
    for k in range(N_CORES):
        sck = np.empty((128, PAIRS * 6), np.float32)
        bik = np.empty((128, PAIRS * 6), np.float32)
        for p in range(PAIRS):
            for i in range(6):
                for ci in range(2):
                    c = k * C_CORE + 2 * p + ci
                    sck[ci * H : (ci + 1) * H, p * 6 + i] = S[i, c]
                    bik[ci * H : (ci + 1) * H, p * 6 + i] = Bv[i, c]
        scbi_cores.append(np.ascontiguousarray(np.concatenate([sck, bik], axis=1)))

    in_maps = [
        {
            "xw": np.ascontiguousarray(
                np.concatenate([xt_cores[k], wm_cores[k]], axis=2)
            ),
            "scbi": scbi_cores[k],
        }
        for k in range(N_CORES)
    ]
    return in_maps


def _assemble(results):
    y = np.empty((B, C, H, W), np.float32)
    for k in range(N_CORES):
        ytk = np.asarray(results[k]["yt"], np.float32)  # [PAIRS, 128, B*W]
        ytk = ytk.reshape(PAIRS, 2, H, B, W).transpose(3, 0, 1, 2, 4)
        y[:, k * C_CORE : (k + 1) * C_CORE] = ytk.reshape(B, C_CORE, H, W)
    return y


def kernel_run(inputs, trace=False, tmpdir=None):
    if "nc" not in _CACHE:
        _CACHE["nc"] = build_nc()
    nc = _CACHE["nc"]
    in_maps = _host_prep(**inputs)
    res = run_bass_kernel_spmd(
        nc, in_maps, list(range(N_CORES)), trace=trace, tmpdir=tmpdir
    )
    return _assemble(res.results), res


def kernel(**inputs):
    out, _ = kernel_run(inputs, trace=False)
    return out


if __name__ == "__main__":
    # smoke test with random data
    rng = np.random.default_rng(0)
    inputs = {
        "x": rng.standard_normal((B, C, H, W), dtype=np.float32),
        "id_bn": np.stack(
            [
                1.0 + 0.1 * rng.standard_normal(C).astype(np.float32),
                0.1 * rng.standard_normal(C).astype(np.float32),
                0.1 * rng.standard_normal(C).astype(np.float32),
                0.5 + rng.random(C).astype(np.float32),
            ]
        ),
        "w5": 0.05 * rng.standard_normal((C, 1, 5, 5), dtype=np.float32),
        "w7": 0.05 * rng.standard_normal((C, 1, 7, 7), dtype=np.float32),
        "w3a": 0.1 * rng.standard_normal((C, 1, 3, 3), dtype=np.float32),
        "w3b": 0.1 * rng.standard_normal((C, 1, 3, 3), dtype=np.float32),
        "w3c": 0.1 * rng.standard_normal((C, 1, 3, 3), dtype=np.float32),
        "bn_gamma": 1.0 + 0.1 * rng.standard_normal((5, C)).astype(np.float32),
        "bn_beta": 0.1 * rng.standard_normal((5, C)).astype(np.float32),
        "bn_mean": 0.1 * rng.standard_normal((5, C)).astype(np.float32),
        "bn_var": 0.5 + rng.random((5, C)).astype(np.float32),
    }
    out = kernel(**inputs)
    print("out", out.shape, out.dtype, float(np.abs(out).max()))
